# revision 2
# baseline (speedup 1.0000x reference)
"""Causal attention (B=2, H=16, L=2048, D=64, fp32) on 8 trn2 NeuronCores.

Sharding: the 32 (batch, head) pairs are split 4-per-core (pure data/head
parallelism, no cross-core comms). Each core runs the same Bass/Tile program
on its own 4 heads (2 head-PAIRS packed on SBUF partitions 0:64 / 64:128).

Device algorithm (per head-pair, per 512-wide q chunk, ascending):
  - Scores TRANSPOSED: S_T[k, q] = sum_d K[k,d] Q[q,d] via
    matmul(lhsT=kT[:, kb*128:+128], rhs=qT[:, chunk]) -> PSUM [128, 1024]
    (1 k-block x 2 heads, f32r full-rate; diagonal blocks causally trimmed
    to >=256-wide so f32r never hits the narrow-moving 4x penalty).
  - exp: no max-subtraction needed (fp32/bf16 range covers |s|<=~88).
    Split across engines by a fixed ratio:
      * ScalarE ACT exp, PSUM -> bf16 SBUF (exact)
      * VectorE one tensor_scalar: i16 = int16(x*128*log2e + 127*128); the
        int16 bits ARE bf16(exp(x)) (Schraudolph). Single instruction.
    Softmax normalization makes the approximation error mostly cancel;
    measured output rel err ~7e-3 with a 0.40 raw share.
  - Causal mask: affine_select (GpSimd) zeroes the upper triangle of the
    diagonal 128x128 blocks of the bf16 exp tiles.
  - PV FLIPPED to out[q, d]: matmul(lhsT=P_T block [k128, q128],
    rhs=V_aug [k128, 65]) accumulating over k-blocks into PSUM [128, 65*4]
    per (head, chunk). V is bf16 with a ones-column appended, so column 64
    is the softmax denominator. 65-wide bf16 matmuls run at 1 cycle/row.
    q-subtiles entirely above the diagonal are skipped.
  - No on-device normalize: PSUM (numerator|denominator) is copied to SBUF
    (VectorE) and DMA'd out; the HOST does out = num/den and the layout
    unshuffle. This removes the reciprocal/broadcast/multiply stages.
"""

import math
import numpy as np
from contextlib import ExitStack

import concourse.bass as bass
import concourse.bacc as bacc
import concourse.mybir as mybir
import concourse.tile as tile
from concourse.bass_utils import run_bass_kernel_spmd

B, H, L, D = 2, 16, 2048, 64
N_CORES = 8
HPC = (B * H) // N_CORES  # heads per core = 4
NPAIR = HPC // 2
QW = 512
NJ = L // QW  # 4 q chunks
GKB = QW // 128  # 4 k-blocks per diagonal group
NKB = L // 128  # 16 k-blocks

F32 = mybir.dt.float32
F32R = mybir.dt.float32r
BF16 = mybir.dt.bfloat16
I16 = mybir.dt.int16
EXP = mybir.ActivationFunctionType.Exp

# Schraudolph constants for direct bf16-bit construction via int16:
# i16 = round(x*log2(e)*2^7 + 127*2^7); bitcast(i16) == bf16(~exp(x))
A16 = float(np.float32(np.log2(math.e) * 2.0**7))
B16 = float(np.float32(127.0 * 2.0**7))

# Fraction of exp columns handled exactly on ScalarE (rest: raw DVE path)
ACT_FRAC = 0.60


def build_nc(act_frac=ACT_FRAC):
    nc = bacc.Bacc(trn_type="TRN2")
    # head-PAIR packed q/k: pair p rows 0:64 = head 2p, rows 64:128 = 2p+1
    qT = nc.dram_tensor("qT", [NPAIR, 2 * D, L], F32R, kind="ExternalInput")
    kT = nc.dram_tensor("kT", [NPAIR, 2 * D, L], F32R, kind="ExternalInput")
    # V augmented with a ones column, bf16
    vA = nc.dram_tensor("vA", [HPC, L, D + 1], BF16, kind="ExternalInput")
    # out: per (head, chunk): [q-part 128, 4 qsubs x (64 num + 1 den)]
    oN = nc.dram_tensor("oN", [HPC, NJ, 128, 4 * (D + 1)], F32,
                        kind="ExternalOutput")

    with tile.TileContext(nc) as tc, ExitStack() as ctx:
        qk_pool = ctx.enter_context(tc.tile_pool(name="qk", bufs=1))
        v_pool = ctx.enter_context(tc.tile_pool(name="v", bufs=1))
        et_pool = ctx.enter_context(tc.tile_pool(name="et", bufs=36))
        out_pool = ctx.enter_context(tc.tile_pool(name="out", bufs=4))
        ps_pool = ctx.enter_context(tc.tile_pool(name="ps", bufs=2, space="PSUM"))
        pv_pool = ctx.enter_context(tc.tile_pool(name="pv", bufs=2, space="PSUM"))

        qts, kts, vas = [None] * NPAIR, [None] * NPAIR, [None] * HPC
        for p in range(NPAIR):
            qts[p] = qk_pool.tile([2 * D, L], F32R, name=f"qt{p}", tag=f"qt{p}")
            kts[p] = qk_pool.tile([2 * D, L], F32R, name=f"kt{p}", tag=f"kt{p}")
        for h in range(HPC):
            vas[h] = v_pool.tile([128, NKB, D + 1], BF16, name=f"va{h}",
                                 tag=f"va{h}")
        # loads: chunk-ascending so compute can start after the first chunk
        for c in range(NJ):
            cs = slice(c * QW, (c + 1) * QW)
            for p in range(NPAIR):
                nc.sync.dma_start(kts[p][:, cs], kT[p][:, cs])
                nc.sync.dma_start(qts[p][:, cs], qT[p][:, cs])
            if c == 0:
                for h in range(HPC):
                    nc.sync.dma_start(
                        vas[h][:],
                        vA[h].rearrange("(ko ki) d -> ki ko d", ki=128),
                    )

        # greedy exp engine balancer (by column count)
        bal = {"act": 0, "tot": 0}

        def pick_engine(cols):
            bal["tot"] += cols
            if bal["act"] < act_frac * bal["tot"]:
                bal["act"] += cols
                return "ACT"
            return "RAW"

        stages = [(j, p) for j in range(NJ) for p in range(NPAIR)]
        # per-stage state: list of (et_tile, trim) per kb
        st_tiles = {}

        def emit_scores(j, p):
            nblk = GKB * (j + 1)
            tiles = []
            for kb in range(nblk):
                o = (kb - GKB * j) * 128 if kb >= GKB * j else None
                trim = 256 if (o is not None and o >= 256) else 0
                w = QW - trim
                ps = ps_pool.tile([128, 2 * QW], F32, name="ps", tag="ps")
                for hh in range(2):
                    nc.tensor.matmul(
                        ps[:, hh * QW: hh * QW + w],
                        lhsT=kts[p][hh * D:(hh + 1) * D, kb * 128:(kb + 1) * 128],
                        rhs=qts[p][hh * D:(hh + 1) * D, j * QW + trim:(j + 1) * QW],
                        start=True,
                        stop=True,
                    )
                et = et_pool.tile([128, 2 * QW], BF16, name="et", tag="et")
                tiles.append((ps, et, o, trim, w))
            st_tiles[(j, p)] = tiles

        def emit_exp_mask(j, p):
            tiles = st_tiles[(j, p)]
            for (ps, et, o, trim, w) in tiles:
                eng = pick_engine(2 * w)
                if trim == 0:
                    ps_ap = ps[:, : 2 * QW]
                    et_ap = et[:, : 2 * QW]
                    ei_ap = et.bitcast(I16)[:, : 2 * QW]
                else:
                    pat = [[2 * QW, 128], [QW, 2], [1, w]]
                    ps_ap = bass.AP(ps.tensor, ps.offset, pat)
                    et_ap = bass.AP(et.tensor, et.offset, pat)
                    ei_ap = bass.AP(et.bitcast(I16).tensor, et.offset, pat)
                if eng == "ACT":
                    nc.scalar.activation(et_ap, ps_ap, EXP)
                else:
                    nc.vector.tensor_scalar(
                        ei_ap, ps_ap, A16, B16,
                        mybir.AluOpType.mult, mybir.AluOpType.add,
                    )
                if o is not None:
                    # zero upper triangle of the diagonal 128x128 block
                    for hh in range(2):
                        tb = hh * QW + (o - trim)
                        sl = et[:, tb: tb + 128]
                        nc.gpsimd.affine_select(
                            out=sl,
                            in_=sl,
                            compare_op=mybir.AluOpType.is_ge,
                            fill=0.0,
                            base=0,
                            pattern=[[1, 128]],
                            channel_multiplier=-1,
                        )

        def emit_pv(j, p):
            tiles = st_tiles.pop((j, p))
            nblk = GKB * (j + 1)
            for hh in range(2):
                h = 2 * p + hh
                pv = pv_pool.tile([128, 4 * (D + 1)], F32, name=f"pv{hh}",
                                  tag=f"pv{hh}")
                for s in range(4):
                    kbs = [kb for kb in range(nblk) if kb <= GKB * j + s]
                    for i, kb in enumerate(kbs):
                        _, et, o, trim, w = tiles[kb]
                        col = hh * QW + s * 128 - trim
                        nc.tensor.matmul(
                            pv[:, s * (D + 1): (s + 1) * (D + 1)],
                            lhsT=et[:, col: col + 128],
                            rhs=vas[h][:, kb, :],
                            start=(i == 0),
                            stop=(i == len(kbs) - 1),
                        )
                ot = out_pool.tile([128, 4 * (D + 1)], F32, name="ot", tag="ot")
                nc.vector.tensor_copy(ot[:], pv[:])
                nc.sync.dma_start(oN[h][j], ot[:])

        emit_scores(*stages[0])
        for i, s in enumerate(stages):
            if i + 1 < len(stages):
                emit_scores(*stages[i + 1])
            emit_exp_mask(*s)
            emit_pv(*s)
    nc.compile()
    return nc


_NC_CACHE = {}


def _get_nc(key=ACT_FRAC):
    if key not in _NC_CACHE:
        _NC_CACHE[key] = build_nc(key)
    return _NC_CACHE[key]


def make_in_maps(q, k, v):
    """Shard + lay out the full [B,H,L,D] inputs into per-core device maps."""
    bf = mybir.dt.np(BF16)
    qf = np.ascontiguousarray(q, dtype=np.float32).reshape(B * H, L, D)
    kf = np.ascontiguousarray(k, dtype=np.float32).reshape(B * H, L, D)
    vf = np.ascontiguousarray(v, dtype=np.float32).reshape(B * H, L, D)
    in_maps = []
    ones = np.ones((HPC, L, 1), dtype=np.float32)
    for c in range(N_CORES):
        sl = slice(HPC * c, HPC * (c + 1))
        qTc = np.ascontiguousarray(qf[sl].transpose(0, 2, 1)).reshape(
            NPAIR, 2 * D, L)
        kTc = np.ascontiguousarray(kf[sl].transpose(0, 2, 1)).reshape(
            NPAIR, 2 * D, L)
        vAc = np.concatenate([vf[sl], ones], axis=2).astype(bf)
        in_maps.append({"qT": qTc, "kT": kTc, "vA": np.ascontiguousarray(vAc)})
    return in_maps


def gather_output(results):
    """Per-core oN [hpc, nj, 128, 4*65] -> full [B, H, L, D] (host divide)."""
    oN = np.concatenate([r["oN"] for r in results], axis=0)  # [B*H, nj,128,260]
    oN = oN.reshape(B * H, NJ, 128, 4, D + 1).transpose(0, 1, 3, 2, 4)
    oN = np.ascontiguousarray(oN).reshape(B * H, L, D + 1)
    out = oN[:, :, :D] / oN[:, :, D:]
    return np.ascontiguousarray(out.reshape(B, H, L, D).astype(np.float32))


def run(q, k, v, trace=False, **spmd_kwargs):
    nc = _get_nc()
    res = run_bass_kernel_spmd(
        nc,
        make_in_maps(q, k, v),
        core_ids=list(range(N_CORES)),
        trace=trace,
        **spmd_kwargs,
    )
    return gather_output(res.results), res


def kernel(q, k, v):
    out, _ = run(q, k, v)
    return out


# revision 27
# speedup vs baseline: 1.7387x; 1.7387x over previous
"""Causal attention (B=2, H=16, L=2048, D=64, fp32) on 8 trn2 NeuronCores.

Sharding: the 32 (batch, head) pairs are split 4-per-core (pure data/head
parallelism, no cross-core comms). Each core runs the same Bass/Tile program
on its own 4 heads (2 head-PAIRS packed on SBUF partitions 0:64 / 64:128).

Device algorithm (per head-pair, per 512-wide q chunk, ascending):
  - Scores TRANSPOSED: S_T[k, q] = sum_d K[k,d] Q[q,d] via
    matmul(lhsT=kT[:, kb*128:+128], rhs=qT[:, chunk]) -> PSUM [128, 1024]
    (1 k-block x 2 heads, f32r full-rate; diagonal blocks causally trimmed
    to >=256-wide so f32r never hits the narrow-moving 4x penalty).
  - exp: no max-subtraction needed (fp32/bf16 range covers |s|<=~88).
    Split across engines by a fixed ratio:
      * ScalarE ACT exp, PSUM -> bf16 SBUF (exact)
      * VectorE one tensor_scalar: i16 = int16(x*128*log2e + 127*128); the
        int16 bits ARE bf16(exp(x)) (Schraudolph). Single instruction.
    Softmax normalization makes the approximation error mostly cancel;
    measured output rel err ~7e-3 with a 0.40 raw share.
  - Causal mask: affine_select (GpSimd) zeroes the upper triangle of the
    diagonal 128x128 blocks of the bf16 exp tiles.
  - PV FLIPPED to out[q, d]: matmul(lhsT=P_T block [k128, q128],
    rhs=V_aug [k128, 65]) accumulating over k-blocks into PSUM [128, 65*4]
    per (head, chunk). V is bf16 with a ones-column appended, so column 64
    is the softmax denominator. 65-wide bf16 matmuls run at 1 cycle/row.
    q-subtiles entirely above the diagonal are skipped.
  - No on-device normalize: PSUM (numerator|denominator) is copied to SBUF
    (VectorE) and DMA'd out; the HOST does out = num/den and the layout
    unshuffle. This removes the reciprocal/broadcast/multiply stages.
"""

import math
import numpy as np
from contextlib import ExitStack

import concourse.bass as bass
import concourse.bacc as bacc
import concourse.mybir as mybir
import concourse.tile as tile
from concourse.bass_utils import run_bass_kernel_spmd

B, H, L, D = 2, 16, 2048, 64
N_CORES = 8
HPC = (B * H) // N_CORES  # heads per core = 4
NPAIR = HPC // 2
QW = 512
NJ = L // QW  # 4 q chunks
GKB = QW // 128  # 4 k-blocks per diagonal group
NKB = L // 128  # 16 k-blocks

F32 = mybir.dt.float32
F32R = mybir.dt.float32r
BF16 = mybir.dt.bfloat16
I16 = mybir.dt.int16
EXP = mybir.ActivationFunctionType.Exp

# Schraudolph constants for direct bf16-bit construction via int16:
# i16 = round(x*log2(e)*2^7 + 127*2^7); bitcast(i16) == bf16(~exp(x))
A16 = float(np.float32(np.log2(math.e) * 2.0**7))
B16 = float(np.float32(127.0 * 2.0**7))

# Fraction of exp columns handled exactly on ScalarE (rest: raw DVE path)
ACT_FRAC = 0.68


def build_nc(act_frac=ACT_FRAC, ps_bufs=3, pv_bufs=1, copies_on="DVE",
             mask_on=True):
    nc = bacc.Bacc(trn_type="TRN2")
    # head-PAIR packed q/k: pair p rows 0:64 = head 2p, rows 64:128 = 2p+1
    qT = nc.dram_tensor("qT", [NPAIR, 2 * D, L], F32R, kind="ExternalInput")
    kT = nc.dram_tensor("kT", [NPAIR, 2 * D, L], F32R, kind="ExternalInput")
    # V augmented with a ones column, bf16, host-pre-shuffled to the SBUF
    # layout [ki=128, ko=16, d] so the load is one contiguous DMA per head
    vA = nc.dram_tensor("vA", [HPC, 128, NKB, D + 1], BF16,
                        kind="ExternalInput")
    # out: per (head, chunk): [q-part 128, 4 qsubs x (64 num + 1 den)]
    oN = nc.dram_tensor("oN", [HPC, NJ, 128, 4 * (D + 1)], F32,
                        kind="ExternalOutput")

    with tile.TileContext(nc) as tc, ExitStack() as ctx:
        qk_pool = ctx.enter_context(tc.tile_pool(name="qk", bufs=1))
        v_pool = ctx.enter_context(tc.tile_pool(name="v", bufs=1))
        et_pool = ctx.enter_context(tc.tile_pool(name="et", bufs=36))
        out_pool = ctx.enter_context(tc.tile_pool(name="out", bufs=4))
        ps_pool = ctx.enter_context(
            tc.tile_pool(name="ps", bufs=ps_bufs, space="PSUM"))
        pv_pool = ctx.enter_context(
            tc.tile_pool(name="pv", bufs=pv_bufs, space="PSUM"))

        qts, kts, vas = [None] * NPAIR, [None] * NPAIR, [None] * HPC
        for p in range(NPAIR):
            qts[p] = qk_pool.tile([2 * D, L], F32R, name=f"qt{p}", tag=f"qt{p}")
            kts[p] = qk_pool.tile([2 * D, L], F32R, name=f"kt{p}", tag=f"kt{p}")
        for h in range(HPC):
            vas[h] = v_pool.tile([128, NKB, D + 1], BF16, name=f"va{h}",
                                 tag=f"va{h}")
        # loads: pair 0's chunks ascending first (it runs first), then vA,
        # then pair 1. The critical chunk-0 k/q go out on separate DGE
        # queues so their transfers + sem-props overlap.
        for c in range(NJ):
            cs = slice(c * QW, (c + 1) * QW)
            if c == 0:
                nc.scalar.dma_start(kts[0][:, cs], kT[0][:, cs])
                nc.sync.dma_start(qts[0][:, cs], qT[0][:, cs])
            else:
                nc.sync.dma_start(kts[0][:, cs], kT[0][:, cs])
                nc.sync.dma_start(qts[0][:, cs], qT[0][:, cs])
            if c == 1:
                for h in range(HPC):
                    nc.sync.dma_start(vas[h][:], vA[h][:])
        for c in range(NJ):
            cs = slice(c * QW, (c + 1) * QW)
            for p in range(1, NPAIR):
                nc.sync.dma_start(kts[p][:, cs], kT[p][:, cs])
                nc.sync.dma_start(qts[p][:, cs], qT[p][:, cs])

        # virtual-clock exp engine balancer: assign each tile to the engine
        # projected to finish it first. act_frac biases the ACT clock rate
        # (lower act_frac -> ACT looks slower -> more tiles go to DVE).
        bal = {"act": 0.0, "dve": 0.0}
        ACT_NS = 0.876 * (0.68 / act_frac)  # ns/col incl. per-instr overhead
        DVE_NS = 1.27

        def pick_engine(cols):
            t_act = bal["act"] + cols * ACT_NS
            t_dve = bal["dve"] + cols * DVE_NS
            if t_act <= t_dve:
                bal["act"] = t_act
                return "ACT"
            bal["dve"] = t_dve
            return "RAW"

        # valley schedule: pair 0 ascending then pair 1 descending, so the
        # pipeline both ramps up and drains on the small j=0 stages
        stages = [(j, 0) for j in range(NJ)] + \
                 [(j, 1) for j in reversed(range(NJ))]
        # per-stage state: list of (et_tile, o, trim, w) per kb
        st_tiles = {}

        def emit_score_exp(j, p, kb):
            """One k-block: score matmuls (2 heads) + exp (+ triangle mask).

            Diagonal blocks (k offset o in the chunk) are causally trimmed:
            the matmul computes q >= trim = min(o, 256) (f32r needs >=256
            moving); exp covers only the valid q >= o columns.
            """
            o = (kb - GKB * j) * 128 if kb >= GKB * j else None
            trim = min(o, 256) if o is not None else 0
            w = QW - trim
            ps = ps_pool.tile([128, 2 * QW], F32, name="ps", tag="ps")
            for hh in range(2):
                nc.tensor.matmul(
                    ps[:, hh * QW: hh * QW + w],
                    lhsT=kts[p][hh * D:(hh + 1) * D, kb * 128:(kb + 1) * 128],
                    rhs=qts[p][hh * D:(hh + 1) * D, j * QW + trim:(j + 1) * QW],
                    start=True,
                    stop=True,
                )
            et = et_pool.tile([128, 2 * QW], BF16, name="et", tag="et")
            # diagonal tiles always go ScalarE (they need the triangle mask;
            # keeps the DVE raw path to plain full-width APs)
            if o is not None:
                wd = QW - o  # only q >= o columns are causally valid
                bal["act"] += 2 * wd * ACT_NS
                eng = "ACT"
                off = o - trim
                pat = [[2 * QW, 128], [QW, 2], [1, wd]]
                ps_ap = bass.AP(ps.tensor, ps.offset + off, pat)
                et_ap = bass.AP(et.tensor, et.offset + off, pat)
                nc.scalar.activation(et_ap, ps_ap, EXP)
            elif pick_engine(2 * QW) == "ACT":
                nc.scalar.activation(et[:, : 2 * QW], ps[:, : 2 * QW], EXP)
            else:
                nc.vector.tensor_scalar(
                    et.bitcast(I16)[:, : 2 * QW], ps[:, : 2 * QW], A16, B16,
                    mybir.AluOpType.mult, mybir.AluOpType.add,
                )
            if o is not None and mask_on:
                for hh in range(2):
                    tb = hh * QW + (o - trim)
                    sl = et[:, tb: tb + 128]
                    nc.gpsimd.affine_select(
                        out=sl,
                        in_=sl,
                        compare_op=mybir.AluOpType.is_ge,
                        fill=0.0,
                        base=0,
                        pattern=[[1, 128]],
                        channel_multiplier=-1,
                    )
            st_tiles.setdefault((j, p), []).append((et, o, trim, w))

        def make_chains(j, p):
            """PV chain closures (one per (head, qsub)) + a copy/DMA flusher."""
            tiles = st_tiles.pop((j, p))
            nblk = GKB * (j + 1)
            chains = []
            pvs = {}

            def chain(hh, s):
                h = 2 * p + hh
                if hh not in pvs:
                    pvs[hh] = pv_pool.tile([128, 4 * (D + 1)], F32,
                                           name=f"pv{hh}", tag=f"pv{hh}")
                pv = pvs[hh]
                kbs = [kb for kb in range(nblk) if kb <= GKB * j + s]
                for i, kb in enumerate(kbs):
                    et, o, trim, w = tiles[kb]
                    col = hh * QW + s * 128 - trim
                    nc.tensor.matmul(
                        pv[:, s * (D + 1): (s + 1) * (D + 1)],
                        lhsT=et[:, col: col + 128],
                        rhs=vas[h][:, kb, :],
                        start=(i == 0),
                        stop=(i == len(kbs) - 1),
                    )

            # s-major: chains with the fewest dependencies (small qsubs need
            # fewer exp'd diagonal tiles) enter the PE queue first
            for s in range(4):
                for hh in range(2):
                    chains.append(lambda hh=hh, s=s: chain(hh, s))

            def flush():
                for hh in range(2):
                    h = 2 * p + hh
                    ot = out_pool.tile([128, 4 * (D + 1)], F32, name="ot",
                                       tag="ot")
                    if copies_on == "ACT":
                        nc.scalar.copy(ot[:], pvs[hh][:])
                        bal["act"] += 500
                    else:
                        nc.vector.tensor_copy(ot[:], pvs[hh][:])
                        bal["dve"] += 500
                    nc.sync.dma_start(oN[h][j], ot[:])

            return chains, flush

        # software pipeline: while stage i+1's scores/exp stream out, the PV
        # chains of stage i are interleaved into the PE queue so PE never
        # head-of-queue-blocks on a not-yet-exp'd score tile.
        pend_chains, pend_flush = [], None
        for j, p in stages:
            nblk = GKB * (j + 1)
            ci = 0
            for kb in range(nblk):
                emit_score_exp(j, p, kb)
                if kb >= 1 and ci < len(pend_chains):
                    pend_chains[ci]()
                    ci += 1
            while ci < len(pend_chains):
                pend_chains[ci]()
                ci += 1
            if pend_flush is not None:
                pend_flush()
            pend_chains, pend_flush = make_chains(j, p)
        for c in pend_chains:
            c()
        pend_flush()
    nc.compile()
    return nc


_NC_CACHE = {}


def _get_nc(key=ACT_FRAC):
    if key not in _NC_CACHE:
        _NC_CACHE[key] = build_nc(key)
    return _NC_CACHE[key]


def make_in_maps(q, k, v):
    """Shard + lay out the full [B,H,L,D] inputs into per-core device maps."""
    bf = mybir.dt.np(BF16)
    qf = np.ascontiguousarray(q, dtype=np.float32).reshape(B * H, L, D)
    kf = np.ascontiguousarray(k, dtype=np.float32).reshape(B * H, L, D)
    vf = np.ascontiguousarray(v, dtype=np.float32).reshape(B * H, L, D)
    in_maps = []
    ones = np.ones((HPC, L, 1), dtype=np.float32)
    for c in range(N_CORES):
        sl = slice(HPC * c, HPC * (c + 1))
        qTc = np.ascontiguousarray(qf[sl].transpose(0, 2, 1)).reshape(
            NPAIR, 2 * D, L)
        kTc = np.ascontiguousarray(kf[sl].transpose(0, 2, 1)).reshape(
            NPAIR, 2 * D, L)
        vAc = np.concatenate([vf[sl], ones], axis=2)  # [hpc, L, 65]
        vAc = vAc.reshape(HPC, NKB, 128, D + 1).transpose(0, 2, 1, 3)
        in_maps.append(
            {"qT": qTc, "kT": kTc,
             "vA": np.ascontiguousarray(vAc).astype(bf)})
    return in_maps


def gather_output(results):
    """Per-core oN [hpc, nj, 128, 4*65] -> full [B, H, L, D] (host divide)."""
    oN = np.concatenate([r["oN"] for r in results], axis=0)  # [B*H, nj,128,260]
    oN = oN.reshape(B * H, NJ, 128, 4, D + 1).transpose(0, 1, 3, 2, 4)
    oN = np.ascontiguousarray(oN).reshape(B * H, L, D + 1)
    out = oN[:, :, :D] / oN[:, :, D:]
    return np.ascontiguousarray(out.reshape(B, H, L, D).astype(np.float32))


def run(q, k, v, trace=False, **spmd_kwargs):
    nc = _get_nc()
    res = run_bass_kernel_spmd(
        nc,
        make_in_maps(q, k, v),
        core_ids=list(range(N_CORES)),
        trace=trace,
        **spmd_kwargs,
    )
    return gather_output(res.results), res


def kernel(q, k, v):
    out, _ = run(q, k, v)
    return out


# revision 32
# speedup vs baseline: 1.8249x; 1.0496x over previous
"""Causal attention (B=2, H=16, L=2048, D=64, fp32) on 8 trn2 NeuronCores.

Sharding: the 32 (batch, head) pairs are split 4-per-core (pure data/head
parallelism, no cross-core comms). Each core runs the same Bass/Tile program
on its own 4 heads (2 head-PAIRS packed on SBUF partitions 0:64 / 64:128).

Device algorithm (per head-pair, per 512-wide q chunk, ascending):
  - Scores TRANSPOSED: S_T[k, q] = sum_d K[k,d] Q[q,d] via
    matmul(lhsT=kT[:, kb*128:+128], rhs=qT[:, chunk]) -> PSUM [128, 1024]
    (1 k-block x 2 heads, f32r full-rate; diagonal blocks causally trimmed
    to >=256-wide so f32r never hits the narrow-moving 4x penalty).
  - exp: no max-subtraction needed (fp32/bf16 range covers |s|<=~88).
    Split across engines by a fixed ratio:
      * ScalarE ACT exp, PSUM -> bf16 SBUF (exact)
      * VectorE one tensor_scalar: i16 = int16(x*128*log2e + 127*128); the
        int16 bits ARE bf16(exp(x)) (Schraudolph). Single instruction.
    Softmax normalization makes the approximation error mostly cancel;
    measured output rel err ~7e-3 with a 0.40 raw share.
  - Causal mask: affine_select (GpSimd) zeroes the upper triangle of the
    diagonal 128x128 blocks of the bf16 exp tiles.
  - PV FLIPPED to out[q, d]: matmul(lhsT=P_T block [k128, q128],
    rhs=V_aug [k128, 65]) accumulating over k-blocks into PSUM [128, 65*4]
    per (head, chunk). V is bf16 with a ones-column appended, so column 64
    is the softmax denominator. 65-wide bf16 matmuls run at 1 cycle/row.
    q-subtiles entirely above the diagonal are skipped.
  - No on-device normalize: PSUM (numerator|denominator) is copied to SBUF
    (VectorE) and DMA'd out; the HOST does out = num/den and the layout
    unshuffle. This removes the reciprocal/broadcast/multiply stages.
"""

import math
import numpy as np
from contextlib import ExitStack

import concourse.bass as bass
import concourse.bacc as bacc
import concourse.mybir as mybir
import concourse.tile as tile
from concourse.bass_utils import run_bass_kernel_spmd

B, H, L, D = 2, 16, 2048, 64
N_CORES = 8
HPC = (B * H) // N_CORES  # heads per core = 4
NPAIR = HPC // 2
QW = 512
NJ = L // QW  # 4 q chunks
GKB = QW // 128  # 4 k-blocks per diagonal group
NKB = L // 128  # 16 k-blocks

F32 = mybir.dt.float32
F32R = mybir.dt.float32r
BF16 = mybir.dt.bfloat16
I16 = mybir.dt.int16
EXP = mybir.ActivationFunctionType.Exp

# Schraudolph constants for direct bf16-bit construction via int16:
# i16 = round(x*log2(e)*2^7 + 127*2^7); bitcast(i16) == bf16(~exp(x))
A16 = float(np.float32(np.log2(math.e) * 2.0**7))
B16 = float(np.float32(127.0 * 2.0**7))

# Fraction of exp columns handled exactly on ScalarE (rest: raw DVE path)
ACT_FRAC = 0.60


def build_nc(act_frac=ACT_FRAC, ps_bufs=3, pv_bufs=1, copies_on="DVE",
             mask_on=True):
    nc = bacc.Bacc(trn_type="TRN2")
    # head-PAIR packed q/k: pair p rows 0:64 = head 2p, rows 64:128 = 2p+1
    qT = nc.dram_tensor("qT", [NPAIR, 2 * D, L], F32R, kind="ExternalInput")
    kT = nc.dram_tensor("kT", [NPAIR, 2 * D, L], F32R, kind="ExternalInput")
    # V augmented with a ones column, bf16, host-pre-shuffled to the SBUF
    # layout [ki=128, ko=16, d] so the load is one contiguous DMA per head
    vA = nc.dram_tensor("vA", [HPC, 128, NKB, D + 1], BF16,
                        kind="ExternalInput")
    # out: per (head, chunk): [q-part 128, 4 qsubs x (64 num + 1 den)]
    oN = nc.dram_tensor("oN", [HPC, NJ, 128, 4 * (D + 1)], F32,
                        kind="ExternalOutput")

    with tile.TileContext(nc) as tc, ExitStack() as ctx:
        qk_pool = ctx.enter_context(tc.tile_pool(name="qk", bufs=1))
        v_pool = ctx.enter_context(tc.tile_pool(name="v", bufs=1))
        et_pool = ctx.enter_context(tc.tile_pool(name="et", bufs=36))
        out_pool = ctx.enter_context(tc.tile_pool(name="out", bufs=4))
        ps_pool = ctx.enter_context(
            tc.tile_pool(name="ps", bufs=ps_bufs, space="PSUM"))
        pv_pool = ctx.enter_context(
            tc.tile_pool(name="pv", bufs=pv_bufs, space="PSUM"))

        qts, kts, vas = [None] * NPAIR, [None] * NPAIR, [None] * HPC
        for p in range(NPAIR):
            qts[p] = qk_pool.tile([2 * D, L], F32R, name=f"qt{p}", tag=f"qt{p}")
            kts[p] = qk_pool.tile([2 * D, L], F32R, name=f"kt{p}", tag=f"kt{p}")
        for h in range(HPC):
            vas[h] = v_pool.tile([128, NKB, D + 1], BF16, name=f"va{h}",
                                 tag=f"va{h}")
        # loads: pair 0's chunks ascending first (it runs first), then vA,
        # then pair 1. The critical chunk-0 k/q go out on separate DGE
        # queues so their transfers + sem-props overlap.
        for c in range(NJ):
            cs = slice(c * QW, (c + 1) * QW)
            if c == 0:
                nc.scalar.dma_start(kts[0][:, cs], kT[0][:, cs])
                nc.sync.dma_start(qts[0][:, cs], qT[0][:, cs])
            else:
                nc.sync.dma_start(kts[0][:, cs], kT[0][:, cs])
                nc.sync.dma_start(qts[0][:, cs], qT[0][:, cs])
            if c == 1:
                for h in range(HPC):
                    nc.sync.dma_start(vas[h][:], vA[h][:])
        for c in range(NJ):
            cs = slice(c * QW, (c + 1) * QW)
            for p in range(1, NPAIR):
                nc.sync.dma_start(kts[p][:, cs], kT[p][:, cs])
                nc.sync.dma_start(qts[p][:, cs], qT[p][:, cs])

        # virtual-clock exp engine balancer: assign each tile to the engine
        # projected to finish it first (cost-model ns). act_frac biases the
        # ACT clock rate (higher act_frac -> ACT looks faster -> gets more).
        bal = {"act": 0.0, "dve": 0.0}
        bias = 0.60 / act_frac

        def pick_engine(cols):
            act_cost = ((cols + 222) / 1.2 + 57) * bias
            dve_cost = (cols + 120) / 0.96 + 70
            if bal["act"] + act_cost <= bal["dve"] + dve_cost:
                bal["act"] += act_cost
                return "ACT"
            bal["dve"] += dve_cost
            return "RAW"

        # valley schedule: pair 0 ascending then pair 1 descending, so the
        # pipeline both ramps up and drains on the small j=0 stages
        stages = [(j, 0) for j in range(NJ)] + \
                 [(j, 1) for j in reversed(range(NJ))]
        # per-stage state: list of (et_tile, o, trim, w) per kb
        st_tiles = {}

        def emit_score_exp(j, p, kb):
            """One k-block: score matmuls (2 heads) + exp (+ triangle mask).

            Diagonal blocks (k offset o in the chunk) are causally trimmed:
            the matmul computes q >= trim = min(o, 256) (f32r needs >=256
            moving); exp covers only the valid q >= o columns.
            """
            o = (kb - GKB * j) * 128 if kb >= GKB * j else None
            trim = min(o, 256) if o is not None else 0
            w = QW - trim
            ps = ps_pool.tile([128, 2 * QW], F32, name="ps", tag="ps")
            for hh in range(2):
                nc.tensor.matmul(
                    ps[:, hh * QW: hh * QW + w],
                    lhsT=kts[p][hh * D:(hh + 1) * D, kb * 128:(kb + 1) * 128],
                    rhs=qts[p][hh * D:(hh + 1) * D, j * QW + trim:(j + 1) * QW],
                    start=True,
                    stop=True,
                )
            et = et_pool.tile([128, 2 * QW], BF16, name="et", tag="et")
            if o is not None:
                # diagonal tile: only q >= o columns are causally valid.
                # ACT: one strided-3D instruction covering both head halves.
                # DVE raw: one plain 2-D slice per head half (the HW rejects
                # strided-3D int16 writes).
                wd = QW - o
                off = o - trim
                act_cost = ((2 * wd + 222) / 1.2 + 57) * bias
                dve_cost = 2 * ((wd + 120) / 0.96 + 70)
                if bal["act"] + act_cost <= bal["dve"] + dve_cost:
                    bal["act"] += act_cost
                    pat = [[2 * QW, 128], [QW, 2], [1, wd]]
                    ps_ap = bass.AP(ps.tensor, ps.offset + off, pat)
                    et_ap = bass.AP(et.tensor, et.offset + off, pat)
                    nc.scalar.activation(et_ap, ps_ap, EXP)
                else:
                    bal["dve"] += dve_cost
                    for hh in range(2):
                        c0 = hh * QW + off
                        nc.vector.tensor_scalar(
                            et.bitcast(I16)[:, c0: c0 + wd],
                            ps[:, c0: c0 + wd], A16, B16,
                            mybir.AluOpType.mult, mybir.AluOpType.add,
                        )
            elif pick_engine(2 * QW) == "ACT":
                nc.scalar.activation(et[:, : 2 * QW], ps[:, : 2 * QW], EXP)
            else:
                nc.vector.tensor_scalar(
                    et.bitcast(I16)[:, : 2 * QW], ps[:, : 2 * QW], A16, B16,
                    mybir.AluOpType.mult, mybir.AluOpType.add,
                )
            if o is not None and mask_on:
                for hh in range(2):
                    tb = hh * QW + (o - trim)
                    sl = et[:, tb: tb + 128]
                    nc.gpsimd.affine_select(
                        out=sl,
                        in_=sl,
                        compare_op=mybir.AluOpType.is_ge,
                        fill=0.0,
                        base=0,
                        pattern=[[1, 128]],
                        channel_multiplier=-1,
                    )
            st_tiles.setdefault((j, p), []).append((et, o, trim, w))

        def make_chains(j, p):
            """PV chain closures (one per (head, qsub)) + a copy/DMA flusher."""
            tiles = st_tiles.pop((j, p))
            nblk = GKB * (j + 1)
            chains = []
            pvs = {}

            def chain(hh, s):
                h = 2 * p + hh
                if hh not in pvs:
                    pvs[hh] = pv_pool.tile([128, 4 * (D + 1)], F32,
                                           name=f"pv{hh}", tag=f"pv{hh}")
                pv = pvs[hh]
                kbs = [kb for kb in range(nblk) if kb <= GKB * j + s]
                for i, kb in enumerate(kbs):
                    et, o, trim, w = tiles[kb]
                    col = hh * QW + s * 128 - trim
                    nc.tensor.matmul(
                        pv[:, s * (D + 1): (s + 1) * (D + 1)],
                        lhsT=et[:, col: col + 128],
                        rhs=vas[h][:, kb, :],
                        start=(i == 0),
                        stop=(i == len(kbs) - 1),
                    )

            # s-major: chains with the fewest dependencies (small qsubs need
            # fewer exp'd diagonal tiles) enter the PE queue first
            for s in range(4):
                for hh in range(2):
                    chains.append(lambda hh=hh, s=s: chain(hh, s))

            def flush():
                for hh in range(2):
                    h = 2 * p + hh
                    ot = out_pool.tile([128, 4 * (D + 1)], F32, name="ot",
                                       tag="ot")
                    use_act = (copies_on == "ACT" or
                               (copies_on == "ALT" and hh == 1))
                    if use_act:
                        nc.scalar.copy(ot[:], pvs[hh][:])
                        bal["act"] += 500
                    else:
                        nc.vector.tensor_copy(ot[:], pvs[hh][:])
                        bal["dve"] += 500
                    nc.sync.dma_start(oN[h][j], ot[:])

            return chains, flush

        # software pipeline: while stage i+1's scores/exp stream out, the PV
        # chains of stage i are interleaved into the PE queue so PE never
        # head-of-queue-blocks on a not-yet-exp'd score tile.
        pend_chains, pend_flush = [], None
        for j, p in stages:
            nblk = GKB * (j + 1)
            ci = 0
            for kb in range(nblk):
                emit_score_exp(j, p, kb)
                if kb >= 1 and ci < len(pend_chains):
                    pend_chains[ci]()
                    ci += 1
            while ci < len(pend_chains):
                pend_chains[ci]()
                ci += 1
            if pend_flush is not None:
                pend_flush()
            pend_chains, pend_flush = make_chains(j, p)
        for c in pend_chains:
            c()
        pend_flush()
    nc.compile()
    return nc


_NC_CACHE = {}


def _get_nc(key=ACT_FRAC):
    if key not in _NC_CACHE:
        _NC_CACHE[key] = build_nc(key)
    return _NC_CACHE[key]


def make_in_maps(q, k, v):
    """Shard + lay out the full [B,H,L,D] inputs into per-core device maps."""
    bf = mybir.dt.np(BF16)
    qf = np.ascontiguousarray(q, dtype=np.float32).reshape(B * H, L, D)
    kf = np.ascontiguousarray(k, dtype=np.float32).reshape(B * H, L, D)
    vf = np.ascontiguousarray(v, dtype=np.float32).reshape(B * H, L, D)
    in_maps = []
    ones = np.ones((HPC, L, 1), dtype=np.float32)
    for c in range(N_CORES):
        sl = slice(HPC * c, HPC * (c + 1))
        qTc = np.ascontiguousarray(qf[sl].transpose(0, 2, 1)).reshape(
            NPAIR, 2 * D, L)
        kTc = np.ascontiguousarray(kf[sl].transpose(0, 2, 1)).reshape(
            NPAIR, 2 * D, L)
        vAc = np.concatenate([vf[sl], ones], axis=2)  # [hpc, L, 65]
        vAc = vAc.reshape(HPC, NKB, 128, D + 1).transpose(0, 2, 1, 3)
        in_maps.append(
            {"qT": qTc, "kT": kTc,
             "vA": np.ascontiguousarray(vAc).astype(bf)})
    return in_maps


def gather_output(results):
    """Per-core oN [hpc, nj, 128, 4*65] -> full [B, H, L, D] (host divide)."""
    oN = np.concatenate([r["oN"] for r in results], axis=0)  # [B*H, nj,128,260]
    oN = oN.reshape(B * H, NJ, 128, 4, D + 1).transpose(0, 1, 3, 2, 4)
    oN = np.ascontiguousarray(oN).reshape(B * H, L, D + 1)
    out = oN[:, :, :D] / oN[:, :, D:]
    return np.ascontiguousarray(out.reshape(B, H, L, D).astype(np.float32))


def run(q, k, v, trace=False, **spmd_kwargs):
    nc = _get_nc()
    res = run_bass_kernel_spmd(
        nc,
        make_in_maps(q, k, v),
        core_ids=list(range(N_CORES)),
        trace=trace,
        **spmd_kwargs,
    )
    return gather_output(res.results), res


def kernel(q, k, v):
    out, _ = run(q, k, v)
    return out
